# revision 6
# baseline (speedup 1.0000x reference)
"""Causal single-head attention (B=4, S=4096, D=1024, Dk=128) on 8 TRN2 NeuronCores.

Sharding: 4 batches x 2 cores/batch. Per batch the 32 causal query blocks
(128 rows) form 16 supertiles of 256 rows; each core runs 8 static "slots"
with key-block capacities [4,8,...,32] (x128 keys). Slot t hosts supertile
j = 2t+1-g (g = core group), so every core executes the identical instruction
graph (SPMD) while the causal workload stays balanced; all per-core variation
(which rows, key positions, padding) is carried by the input data.

On-chip layout is fully transposed: host supplies x^T (bf16), projections give
Q^T/K^T/V^T with dk on partitions, scores are computed as S^T = K^T.T @ Q^T so
the softmax'd tile is already P^T = exp(S^T)*mask, which feeds P@V directly
(lhsT = V block, rhs = P^T) with zero per-block transposes. Softmax runs
without max-subtraction (inputs are unit-scale Gaussians; masked entries are
killed multiplicatively post-exp). Key padding is folded into V rows and the
row-sum column; dead query rows are zeroed via the final normalization scale.
"""

import numpy as np
import ml_dtypes

import concourse.bass as bass
import concourse.mybir as mybir
import concourse.tile as tile
from concourse import bacc
from concourse.bass_utils import run_bass_kernel_spmd
from concourse.masks import make_identity

F32 = mybir.dt.float32
F32R = mybir.dt.float32r
BF16 = mybir.dt.bfloat16
AF = mybir.ActivationFunctionType
ALU = mybir.AluOpType

B, S, D, DK = 4, 4096, 1024, 128
NSLOT = 8          # static slots per core
STR = 256          # supertile rows (2 query blocks)
NKB = 32           # key blocks per batch
NCORE = 8
SCALE = float(1.0 / np.sqrt(np.float32(DK)))
BF = ml_dtypes.bfloat16


def build_graph():
    nc = bacc.Bacc("TRN2", target_bir_lowering=False, debug=False, num_devices=NCORE)

    xt_e = nc.declare_dram_parameter("xt", [128, 8, S], BF16, isOutput=False)
    wq_e = nc.declare_dram_parameter("wq", [128, 8, DK], BF16, isOutput=False)
    wk_e = nc.declare_dram_parameter("wk", [128, 8, DK], BF16, isOutput=False)
    wv_e = nc.declare_dram_parameter("wv", [128, 8, DK], BF16, isOutput=False)
    bq_e = nc.declare_dram_parameter("bq", [128, 1], F32, isOutput=False)
    bk_e = nc.declare_dram_parameter("bk", [128, 1], F32, isOutput=False)
    bv_e = nc.declare_dram_parameter("bv", [128, 1], F32, isOutput=False)
    qpos_e = nc.declare_dram_parameter("qpos", [1, NSLOT, STR], F32, isOutput=False)
    pmq_e = nc.declare_dram_parameter("pmq", [1, NSLOT, STR], F32, isOutput=False)
    omp_e = nc.declare_dram_parameter("omp", [1, NSLOT, STR], F32, isOutput=False)
    kposm_e = nc.declare_dram_parameter("kposm", [128, NSLOT, 4], F32, isOutput=False)
    pmk_e = nc.declare_dram_parameter("pmk", [128, NKB], BF16, isOutput=False)
    pmkf_e = nc.declare_dram_parameter("pmkf", [128, NKB], F32, isOutput=False)
    out_e = nc.declare_dram_parameter("out", [NSLOT, STR, DK], F32, isOutput=True)

    with tile.TileContext(nc) as tc:
        with (
            tc.tile_pool(name="const", bufs=1) as const,
            tc.tile_pool(name="big", bufs=1) as big,
            tc.tile_pool(name="vtmp", bufs=2) as vtmp,
            tc.tile_pool(name="pt", bufs=4) as ptp,
            tc.tile_pool(name="sm", bufs=2) as sm,
            tc.tile_pool(name="ot", bufs=2) as otp,
            tc.tile_pool(name="osb", bufs=2) as osbp,
            tc.tile_pool(name="proj_ps", bufs=2, space="PSUM") as proj_ps,
            tc.tile_pool(name="s_ps", bufs=2, space="PSUM") as s_ps,
            tc.tile_pool(name="o_ps", bufs=1, space="PSUM") as o_ps,
            tc.tile_pool(name="l_ps", bufs=1, space="PSUM") as l_ps,
            tc.tile_pool(name="tp_ps", bufs=2, space="PSUM") as tp_ps,
        ):
            # ---- constants / small inputs ----
            ident = const.tile([128, 128], F32)
            make_identity(nc, ident)
            wq_sb = const.tile([128, 8, DK], BF16)
            wk_sb = const.tile([128, 8, DK], BF16)
            wv_sb = const.tile([128, 8, DK], BF16)
            nc.sync.dma_start(wq_sb[:], wq_e[:])
            nc.sync.dma_start(wk_sb[:], wk_e[:])
            nc.sync.dma_start(wv_sb[:], wv_e[:])
            bq_sb = const.tile([128, 1], F32)
            bk_sb = const.tile([128, 1], F32)
            bv_sb = const.tile([128, 1], F32)
            nc.sync.dma_start(bq_sb[:], bq_e[:])
            nc.sync.dma_start(bk_sb[:], bk_e[:])
            nc.sync.dma_start(bv_sb[:], bv_e[:])
            qpos_sb = const.tile([1, NSLOT, STR], F32)
            pmq_sb = const.tile([1, NSLOT, STR], F32)
            omp_sb = const.tile([1, NSLOT, STR], F32)
            nc.sync.dma_start(qpos_sb[:], qpos_e[:])
            nc.sync.dma_start(pmq_sb[:], pmq_e[:])
            nc.sync.dma_start(omp_sb[:], omp_e[:])
            kposm_sb = const.tile([128, NSLOT, 4], F32)
            nc.sync.dma_start(kposm_sb[:], kposm_e[:])
            pmk_sb = const.tile([128, NKB], BF16)
            nc.sync.dma_start(pmk_sb[:], pmk_e[:])
            pmkf_sb = const.tile([128, NKB], F32)
            nc.sync.dma_start(pmkf_sb[:], pmkf_e[:])

            # ---- big SBUF residents ----
            xt_sb = big.tile([128, 8, S], BF16)
            kt_sb = big.tile([128, NKB, DK], F32R)
            v_sb = big.tile([128, NKB, DK], BF16)
            qt_sb = big.tile([128, NSLOT, STR], F32R)
            mask_sb = big.tile([128, NSLOT, 4, STR], BF16)
            qpos_bc = big.tile([128, NSLOT, STR], F32)

            # masks: (key_pos <= query_pos) as 0/1 bf16, per slot for the
            # last 4 key blocks of that slot
            for st in range(NSLOT):
                nc.gpsimd.partition_broadcast(
                    qpos_bc[:, st, :], qpos_sb[0:1, st, :]
                )
            for st in range(NSLOT):
                for i in range(4):
                    nc.vector.tensor_tensor(
                        mask_sb[:, st, i, :],
                        kposm_sb[:, st, i : i + 1].to_broadcast([128, STR]),
                        qpos_bc[:, st, :],
                        ALU.is_le,
                    )

            # interleave projections (chunk c = 512 keys = 4 key blocks)
            # with attention slots (slot st consumes chunks 0..st)
            for c in range(8):
                nc.sync.dma_start(
                    xt_sb[:, :, 512 * c : 512 * (c + 1)],
                    xt_e[:, :, 512 * c : 512 * (c + 1)],
                )
                # K^T chunk -> kt_sb fp32 (+bias)
                kps = proj_ps.tile([128, 512], F32, tag="proj")
                for mc in range(8):
                    nc.tensor.matmul(
                        kps,
                        lhsT=wk_sb[:, mc],
                        rhs=xt_sb[:, mc, 512 * c : 512 * (c + 1)],
                        start=(mc == 0),
                        stop=(mc == 7),
                    )
                nc.vector.tensor_tensor(
                    kt_sb[:, 4 * c : 4 * (c + 1), :],
                    kps,
                    bk_sb[:].to_broadcast([128, 512]),
                    ALU.add,
                )
                # V^T chunk -> tmp fp32 (+bias), then transpose to V rows,
                # scaled by key padding, stored bf16
                vps = proj_ps.tile([128, 512], F32, tag="proj")
                for mc in range(8):
                    nc.tensor.matmul(
                        vps,
                        lhsT=wv_sb[:, mc],
                        rhs=xt_sb[:, mc, 512 * c : 512 * (c + 1)],
                        start=(mc == 0),
                        stop=(mc == 7),
                    )
                vt_sb = vtmp.tile([128, 512], F32, tag="vt")
                nc.vector.tensor_tensor(
                    vt_sb, vps, bv_sb[:].to_broadcast([128, 512]), ALU.add
                )
                for i in range(4):
                    kb = 4 * c + i
                    tps = tp_ps.tile([128, 128], F32, tag="tp")
                    nc.tensor.transpose(tps, vt_sb[:, 128 * i : 128 * (i + 1)], ident)
                    nc.vector.tensor_tensor(
                        v_sb[:, kb, :],
                        tps,
                        pmkf_sb[:, kb : kb + 1].to_broadcast([128, DK]),
                        ALU.mult,
                    )
                # Q^T for slot c (own rows are the 2nd half of each chunk)
                qps = proj_ps.tile([128, 512], F32, tag="proj")
                for mc in range(8):
                    nc.tensor.matmul(
                        qps[:, :STR],
                        lhsT=wq_sb[:, mc],
                        rhs=xt_sb[:, mc, 512 * c + STR : 512 * (c + 1)],
                        start=(mc == 0),
                        stop=(mc == 7),
                    )
                nc.vector.tensor_tensor(
                    qt_sb[:, c, :],
                    qps[:, :STR],
                    bq_sb[:].to_broadcast([128, STR]),
                    ALU.add,
                )

                # ---- attention slot st = c ----
                st = c
                nkb = 4 * st + 4
                ops = o_ps.tile([128, STR], F32, tag="o")
                lps = l_ps.tile([1, STR], F32, tag="l")
                for kb in range(nkb):
                    sps = s_ps.tile([128, STR], F32, tag="s")
                    nc.tensor.matmul(
                        sps,
                        lhsT=kt_sb[:, kb, :],
                        rhs=qt_sb[:, st, :],
                        start=True,
                        stop=True,
                    )
                    pt = ptp.tile([128, STR], BF16, tag="pt")
                    nc.scalar.activation(pt, sps, AF.Exp, scale=SCALE)
                    if kb >= nkb - 4:
                        nc.vector.tensor_tensor(
                            pt, pt, mask_sb[:, st, kb - (nkb - 4), :], ALU.mult
                        )
                    nc.tensor.matmul(
                        ops,
                        lhsT=v_sb[:, kb, :],
                        rhs=pt,
                        start=(kb == 0),
                        stop=(kb == nkb - 1),
                    )
                    nc.tensor.matmul(
                        lps,
                        lhsT=pmk_sb[:, kb : kb + 1],
                        rhs=pt,
                        start=(kb == 0),
                        stop=(kb == nkb - 1),
                    )
                # normalization scale: pmq / (l + (1 - pmq)), broadcast over dv
                l_sb = sm.tile([1, STR], F32, tag="lsb")
                nc.vector.tensor_tensor(l_sb, lps, omp_sb[0:1, st, :], ALU.add)
                nc.vector.reciprocal(l_sb, l_sb)
                nc.vector.tensor_tensor(l_sb, l_sb, pmq_sb[0:1, st, :], ALU.mult)
                sc_bc = sm.tile([128, STR], F32, tag="scbc")
                nc.gpsimd.partition_broadcast(sc_bc, l_sb)
                ot_sb = otp.tile([128, STR], F32, tag="ot")
                nc.vector.tensor_tensor(ot_sb, ops, sc_bc, ALU.mult)
                # O^T -> O, DMA out
                for i in range(2):
                    tps = tp_ps.tile([128, 128], F32, tag="tp")
                    nc.tensor.transpose(tps, ot_sb[:, 128 * i : 128 * (i + 1)], ident)
                    o_sb = osbp.tile([128, 128], F32, tag="osb")
                    nc.vector.tensor_copy(o_sb, tps)
                    nc.sync.dma_start(out_e[st, 128 * i : 128 * (i + 1), :], o_sb)

    nc.compile()
    return nc


def shard_inputs(x, padding_mask, Wq, bq, Wk, bk, Wv, bv):
    """Build per-core in_maps plus the scatter info for gathering."""
    x = np.asarray(x, np.float32)
    pm = np.asarray(padding_mask, np.float32)
    w_tiles = {}
    for name, W in (("wq", Wq), ("wk", Wk), ("wv", Wv)):
        w_tiles[name] = np.ascontiguousarray(
            np.asarray(W, np.float32).reshape(8, 128, DK).transpose(1, 0, 2)
        ).astype(BF)
    biases = {
        "bq": np.asarray(bq, np.float32).reshape(128, 1),
        "bk": np.asarray(bk, np.float32).reshape(128, 1),
        "bv": np.asarray(bv, np.float32).reshape(128, 1),
    }
    in_maps = []
    row_maps = []
    base = np.arange(S).reshape(8, 2, STR)
    for c in range(NCORE):
        b, g = c % 4, c // 4
        perm = (base[:, ::-1, :] if g == 1 else base).reshape(-1)
        xp = x[b][perm]                       # [S, D] permuted rows
        xt = np.ascontiguousarray(
            xp.T.reshape(8, 128, S).transpose(1, 0, 2)
        ).astype(BF)
        qrows = perm.reshape(8, 2, STR)[:, 1, :]   # own rows per slot [8, 256]
        qpos = qrows.astype(np.float32)[None]      # [1, 8, 256]
        pmq = pm[b][qrows][None].astype(np.float32)
        kposm = np.zeros((128, NSLOT, 4), np.float32)
        for st in range(NSLOT):
            for i in range(4):
                kb = 4 * st + i
                kposm[:, st, i] = perm[kb * 128 : (kb + 1) * 128]
        pmk = pm[b][perm].reshape(NKB, 128).T      # [128, 32]
        in_maps.append({
            "xt": xt,
            **w_tiles,
            **biases,
            "qpos": np.ascontiguousarray(qpos),
            "pmq": np.ascontiguousarray(pmq),
            "omp": np.ascontiguousarray(1.0 - pmq),
            "kposm": kposm,
            "pmk": np.ascontiguousarray(pmk).astype(BF),
            "pmkf": np.ascontiguousarray(pmk, np.float32),
        })
        row_maps.append((b, qrows))
    return in_maps, row_maps


def gather_outputs(results, row_maps):
    full = np.zeros((B, S, DK), np.float32)
    for c in range(NCORE):
        b, qrows = row_maps[c]
        out = np.asarray(results[c]["out"], np.float32)  # [8, 256, 128]
        for st in range(NSLOT):
            full[b, qrows[st]] = out[st]
    return full


_NC_CACHE = None


def _get_graph():
    global _NC_CACHE
    if _NC_CACHE is None:
        _NC_CACHE = build_graph()
    return _NC_CACHE


def kernel(x, padding_mask, Wq, bq, Wk, bk, Wv, bv):
    nc = _get_graph()
    in_maps, row_maps = shard_inputs(x, padding_mask, Wq, bq, Wk, bk, Wv, bv)
    res = run_bass_kernel_spmd(nc, in_maps, core_ids=list(range(NCORE)))
    return gather_outputs(res.results, row_maps)


# revision 9
# speedup vs baseline: 1.1045x; 1.1045x over previous
"""Causal single-head attention (B=4, S=4096, D=1024, Dk=128) on 8 TRN2 NeuronCores.

Sharding: 4 batches x 2 cores/batch. Per batch the 32 causal query blocks
(128 rows) form 16 supertiles of 256 rows; each core runs 8 static "slots"
with key-block capacities [4,8,...,32] (x128 keys). Slot t hosts supertile
j = 2t+1-g (g = core group), so every core executes the identical instruction
graph (SPMD) while the causal workload stays balanced; all per-core variation
(which rows, key positions, padding) is carried by the input data.

On-chip layout is fully transposed: host supplies x^T (bf16), projections give
Q^T/K^T/V^T with dk on partitions, scores are computed as S^T = K^T.T @ Q^T so
the softmax'd tile is already P^T = exp(S^T)*mask, which feeds P@V directly
(lhsT = V block, rhs = P^T) with zero per-block transposes. Softmax runs
without max-subtraction (inputs are unit-scale Gaussians; masked entries are
killed multiplicatively post-exp). Key padding is folded into V rows and the
row-sum column; dead query rows are zeroed via the final normalization scale.
"""

import numpy as np
import ml_dtypes

import concourse.bass as bass
import concourse.mybir as mybir
import concourse.tile as tile
from concourse import bacc
from concourse.bass_utils import run_bass_kernel_spmd
from concourse.masks import make_identity

F32 = mybir.dt.float32
F32R = mybir.dt.float32r
BF16 = mybir.dt.bfloat16
AF = mybir.ActivationFunctionType
ALU = mybir.AluOpType

B, S, D, DK = 4, 4096, 1024, 128
NSLOT = 8          # static slots per core
STR = 256          # supertile rows (2 query blocks)
NKB = 32           # key blocks per batch
NCORE = 8
SCALE = float(1.0 / np.sqrt(np.float32(DK)))
BF = ml_dtypes.bfloat16


def build_graph():
    nc = bacc.Bacc("TRN2", target_bir_lowering=False, debug=False, num_devices=NCORE)

    xt_e = nc.declare_dram_parameter("xt", [128, 8, S], BF16, isOutput=False)
    wq_e = nc.declare_dram_parameter("wq", [128, 8, DK], BF16, isOutput=False)
    wk_e = nc.declare_dram_parameter("wk", [128, 8, DK], BF16, isOutput=False)
    wv_e = nc.declare_dram_parameter("wv", [128, 8, DK], BF16, isOutput=False)
    bq_e = nc.declare_dram_parameter("bq", [128, 1], F32, isOutput=False)
    bk_e = nc.declare_dram_parameter("bk", [128, 1], F32, isOutput=False)
    bv_e = nc.declare_dram_parameter("bv", [128, 1], F32, isOutput=False)
    qpos_e = nc.declare_dram_parameter("qpos", [1, NSLOT, STR], F32, isOutput=False)
    pmq_e = nc.declare_dram_parameter("pmq", [1, NSLOT, STR], F32, isOutput=False)
    omp_e = nc.declare_dram_parameter("omp", [1, NSLOT, STR], F32, isOutput=False)
    kposm_e = nc.declare_dram_parameter("kposm", [128, NSLOT, 4], F32, isOutput=False)
    pmk_e = nc.declare_dram_parameter("pmk", [128, NKB], BF16, isOutput=False)
    pmkf_e = nc.declare_dram_parameter("pmkf", [128, NKB], F32, isOutput=False)
    out_e = nc.declare_dram_parameter("out", [NSLOT, STR, DK], F32, isOutput=True)

    with tile.TileContext(nc) as tc:
        with (
            tc.tile_pool(name="const", bufs=1) as const,
            tc.tile_pool(name="big", bufs=1) as big,
            tc.tile_pool(name="vtmp", bufs=2) as vtmp,
            tc.tile_pool(name="pt", bufs=4) as ptp,
            tc.tile_pool(name="sm", bufs=2) as sm,
            tc.tile_pool(name="ot", bufs=2) as otp,
            tc.tile_pool(name="osb", bufs=2) as osbp,
            tc.tile_pool(name="proj_ps", bufs=2, space="PSUM") as proj_ps,
            tc.tile_pool(name="s_ps", bufs=2, space="PSUM") as s_ps,
            tc.tile_pool(name="o_ps", bufs=1, space="PSUM") as o_ps,
            tc.tile_pool(name="l_ps", bufs=1, space="PSUM") as l_ps,
            tc.tile_pool(name="tp_ps", bufs=2, space="PSUM") as tp_ps,
        ):
            # ---- constants / small inputs ----
            ident = const.tile([128, 128], F32)
            make_identity(nc, ident)
            wq_sb = const.tile([128, 8, DK], BF16)
            wk_sb = const.tile([128, 8, DK], BF16)
            wv_sb = const.tile([128, 8, DK], BF16)
            nc.sync.dma_start(wq_sb[:], wq_e[:])
            nc.sync.dma_start(wk_sb[:], wk_e[:])
            nc.sync.dma_start(wv_sb[:], wv_e[:])
            bq_sb = const.tile([128, 1], F32)
            bk_sb = const.tile([128, 1], F32)
            bv_sb = const.tile([128, 1], F32)
            nc.sync.dma_start(bq_sb[:], bq_e[:])
            nc.sync.dma_start(bk_sb[:], bk_e[:])
            nc.sync.dma_start(bv_sb[:], bv_e[:])
            qpos_sb = const.tile([1, NSLOT, STR], F32)
            pmq_sb = const.tile([1, NSLOT, STR], F32)
            omp_sb = const.tile([1, NSLOT, STR], F32)
            nc.sync.dma_start(qpos_sb[:], qpos_e[:])
            nc.sync.dma_start(pmq_sb[:], pmq_e[:])
            nc.sync.dma_start(omp_sb[:], omp_e[:])
            kposm_sb = const.tile([128, NSLOT, 4], F32)
            nc.sync.dma_start(kposm_sb[:], kposm_e[:])
            pmk_sb = const.tile([128, NKB], BF16)
            nc.sync.dma_start(pmk_sb[:], pmk_e[:])
            pmkf_sb = const.tile([128, NKB], F32)
            nc.sync.dma_start(pmkf_sb[:], pmkf_e[:])

            # ---- big SBUF residents ----
            xt_sb = big.tile([128, 8, S], BF16)
            kt_sb = big.tile([128, NKB, DK], BF16)
            v_sb = big.tile([128, NKB, DK], BF16)
            qt_sb = big.tile([128, NSLOT, STR], BF16)
            mask_sb = big.tile([128, NSLOT, 4, STR], BF16)
            qpos_bc = big.tile([128, NSLOT, STR], F32)

            # masks: (key_pos <= query_pos) as 0/1 bf16, per slot for the
            # last 4 key blocks of that slot
            for st in range(NSLOT):
                nc.gpsimd.partition_broadcast(
                    qpos_bc[:, st, :], qpos_sb[0:1, st, :]
                )
            for st in range(NSLOT):
                for i in range(4):
                    nc.vector.tensor_tensor(
                        mask_sb[:, st, i, :],
                        kposm_sb[:, st, i : i + 1].to_broadcast([128, STR]),
                        qpos_bc[:, st, :],
                        ALU.is_le,
                    )

            # interleave projections (chunk c = 512 keys = 4 key blocks)
            # with attention slots (slot st consumes chunks 0..st)
            for c in range(8):
                nc.sync.dma_start(
                    xt_sb[:, :, 512 * c : 512 * (c + 1)],
                    xt_e[:, :, 512 * c : 512 * (c + 1)],
                )
                # K^T chunk -> kt_sb fp32 (+bias)
                kps = proj_ps.tile([128, 512], F32, tag="proj")
                for mc in range(8):
                    nc.tensor.matmul(
                        kps,
                        lhsT=wk_sb[:, mc],
                        rhs=xt_sb[:, mc, 512 * c : 512 * (c + 1)],
                        start=(mc == 0),
                        stop=(mc == 7),
                    )
                nc.vector.tensor_tensor(
                    kt_sb[:, 4 * c : 4 * (c + 1), :],
                    kps,
                    bk_sb[:].to_broadcast([128, 512]),
                    ALU.add,
                )
                # V^T chunk -> tmp fp32 (+bias), then transpose to V rows,
                # scaled by key padding, stored bf16
                vps = proj_ps.tile([128, 512], F32, tag="proj")
                for mc in range(8):
                    nc.tensor.matmul(
                        vps,
                        lhsT=wv_sb[:, mc],
                        rhs=xt_sb[:, mc, 512 * c : 512 * (c + 1)],
                        start=(mc == 0),
                        stop=(mc == 7),
                    )
                vt_sb = vtmp.tile([128, 512], F32, tag="vt")
                nc.vector.tensor_tensor(
                    vt_sb, vps, bv_sb[:].to_broadcast([128, 512]), ALU.add
                )
                for i in range(4):
                    kb = 4 * c + i
                    tps = tp_ps.tile([128, 128], F32, tag="tp")
                    nc.tensor.transpose(tps, vt_sb[:, 128 * i : 128 * (i + 1)], ident)
                    nc.vector.tensor_tensor(
                        v_sb[:, kb, :],
                        tps,
                        pmkf_sb[:, kb : kb + 1].to_broadcast([128, DK]),
                        ALU.mult,
                    )
                # Q^T for slot c (own rows are the 2nd half of each chunk)
                qps = proj_ps.tile([128, 512], F32, tag="proj")
                for mc in range(8):
                    nc.tensor.matmul(
                        qps[:, :STR],
                        lhsT=wq_sb[:, mc],
                        rhs=xt_sb[:, mc, 512 * c + STR : 512 * (c + 1)],
                        start=(mc == 0),
                        stop=(mc == 7),
                    )
                nc.vector.tensor_tensor(
                    qt_sb[:, c, :],
                    qps[:, :STR],
                    bq_sb[:].to_broadcast([128, STR]),
                    ALU.add,
                )

                # ---- attention slot st = c ----
                st = c
                nkb = 4 * st + 4
                ops = o_ps.tile([128, STR], F32, tag="o")
                lps = l_ps.tile([1, STR], F32, tag="l")
                for kb in range(nkb):
                    sps = s_ps.tile([128, STR], F32, tag="s")
                    nc.tensor.matmul(
                        sps,
                        lhsT=kt_sb[:, kb, :],
                        rhs=qt_sb[:, st, :],
                        start=True,
                        stop=True,
                    )
                    pt = ptp.tile([128, STR], BF16, tag="pt")
                    nc.scalar.activation(pt, sps, AF.Exp, scale=SCALE)
                    if kb >= nkb - 4:
                        nc.vector.tensor_tensor(
                            pt, pt, mask_sb[:, st, kb - (nkb - 4), :], ALU.mult
                        )
                    nc.tensor.matmul(
                        ops,
                        lhsT=v_sb[:, kb, :],
                        rhs=pt,
                        start=(kb == 0),
                        stop=(kb == nkb - 1),
                    )
                    nc.tensor.matmul(
                        lps,
                        lhsT=pmk_sb[:, kb : kb + 1],
                        rhs=pt,
                        start=(kb == 0),
                        stop=(kb == nkb - 1),
                    )
                # normalization scale: pmq / (l + (1 - pmq)), broadcast over dv
                l_sb = sm.tile([1, STR], F32, tag="lsb")
                nc.vector.tensor_tensor(l_sb, lps, omp_sb[0:1, st, :], ALU.add)
                nc.vector.reciprocal(l_sb, l_sb)
                nc.vector.tensor_tensor(l_sb, l_sb, pmq_sb[0:1, st, :], ALU.mult)
                sc_bc = sm.tile([128, STR], F32, tag="scbc")
                nc.gpsimd.partition_broadcast(sc_bc, l_sb)
                ot_sb = otp.tile([128, STR], F32, tag="ot")
                nc.vector.tensor_tensor(ot_sb, ops, sc_bc, ALU.mult)
                # O^T -> O, DMA out
                for i in range(2):
                    tps = tp_ps.tile([128, 128], F32, tag="tp")
                    nc.tensor.transpose(tps, ot_sb[:, 128 * i : 128 * (i + 1)], ident)
                    o_sb = osbp.tile([128, 128], F32, tag="osb")
                    nc.vector.tensor_copy(o_sb, tps)
                    nc.sync.dma_start(out_e[st, 128 * i : 128 * (i + 1), :], o_sb)

    nc.compile()
    return nc


def shard_inputs(x, padding_mask, Wq, bq, Wk, bk, Wv, bv):
    """Build per-core in_maps plus the scatter info for gathering."""
    x = np.asarray(x, np.float32)
    pm = np.asarray(padding_mask, np.float32)
    w_tiles = {}
    for name, W in (("wq", Wq), ("wk", Wk), ("wv", Wv)):
        w_tiles[name] = np.ascontiguousarray(
            np.asarray(W, np.float32).reshape(8, 128, DK).transpose(1, 0, 2)
        ).astype(BF)
    biases = {
        "bq": np.asarray(bq, np.float32).reshape(128, 1),
        "bk": np.asarray(bk, np.float32).reshape(128, 1),
        "bv": np.asarray(bv, np.float32).reshape(128, 1),
    }
    in_maps = []
    row_maps = []
    base = np.arange(S).reshape(8, 2, STR)
    for c in range(NCORE):
        b, g = c % 4, c // 4
        perm = (base[:, ::-1, :] if g == 1 else base).reshape(-1)
        xp = x[b][perm]                       # [S, D] permuted rows
        xt = np.ascontiguousarray(
            xp.T.reshape(8, 128, S).transpose(1, 0, 2)
        ).astype(BF)
        qrows = perm.reshape(8, 2, STR)[:, 1, :]   # own rows per slot [8, 256]
        qpos = qrows.astype(np.float32)[None]      # [1, 8, 256]
        pmq = pm[b][qrows][None].astype(np.float32)
        kposm = np.zeros((128, NSLOT, 4), np.float32)
        for st in range(NSLOT):
            for i in range(4):
                kb = 4 * st + i
                kposm[:, st, i] = perm[kb * 128 : (kb + 1) * 128]
        pmk = pm[b][perm].reshape(NKB, 128).T      # [128, 32]
        in_maps.append({
            "xt": xt,
            **w_tiles,
            **biases,
            "qpos": np.ascontiguousarray(qpos),
            "pmq": np.ascontiguousarray(pmq),
            "omp": np.ascontiguousarray(1.0 - pmq),
            "kposm": kposm,
            "pmk": np.ascontiguousarray(pmk).astype(BF),
            "pmkf": np.ascontiguousarray(pmk, np.float32),
        })
        row_maps.append((b, qrows))
    return in_maps, row_maps


def gather_outputs(results, row_maps):
    full = np.zeros((B, S, DK), np.float32)
    for c in range(NCORE):
        b, qrows = row_maps[c]
        out = np.asarray(results[c]["out"], np.float32)  # [8, 256, 128]
        for st in range(NSLOT):
            full[b, qrows[st]] = out[st]
    return full


_NC_CACHE = None


def _get_graph():
    global _NC_CACHE
    if _NC_CACHE is None:
        _NC_CACHE = build_graph()
    return _NC_CACHE


def kernel(x, padding_mask, Wq, bq, Wk, bk, Wv, bv):
    nc = _get_graph()
    in_maps, row_maps = shard_inputs(x, padding_mask, Wq, bq, Wk, bk, Wv, bv)
    res = run_bass_kernel_spmd(nc, in_maps, core_ids=list(range(NCORE)))
    return gather_outputs(res.results, row_maps)
